# revision 1
# baseline (speedup 1.0000x reference)
"""MultiHeadAttention (B=2, S=2048, D=1024, H=16) on 8 TRN2 NeuronCores.

Sharding: core c -> batch b = c//4, head-group g = c%4 (4 heads = 256 channels).
Each core computes its 4 heads' attention for its batch plus the partial
out-projection (out_w columns for its channel group); host sums the 4 partials
per batch and adds out_b.

Dtypes: all matmul operands are 2-byte (fp16, except exp outputs / denominator
rows in bf16 for range) — fp32r keeps the same 1 cycle/row on the PE but draws
~2x the SBUF-read + datapath power, and the fp32r baseline sat at the DEC
throttle's lowest p-state (~0.69 GHz effective vs 2.4 peak) for the whole
attention phase; fp16 sustains ~1.2 GHz. fp8 does NOT help further (probed:
fp8 matmuls run at exactly fp16 speed — the sustained clamp is duty-based and
dtype-blind below 2 bytes). Accumulation stays fp32 in PSUM; measured rel err
2.9e-3 (vs 8.8e-4 all-fp32r; harness gate 2e-2).

Schedule notes (evidence from NTFF traces, worth ~50us over the naive order):
 - kc loop software-pipelined one stage (AV matmuls for kc-1 after logits
   matmuls for kc) so AV never stalls on the ACT exps.
 - One DMA descriptor per weight tensor (3D-AP, 512B runs) — per-issue cost
   on the sync queue is ~600ns; NOTE a host-side pre-shuffle to contiguous
   [128, DC*JG] DMAs measured ~18us SLOWER than the strided descriptors.
 - Phase-A PSUM tag sets alternate per group; denominator-row copies both on
   DVE (an ACT copy makes the next pair's logits WAR-wait the ACT counter).
 - Timing is thermally sensitive (~60us swings back-to-back); compare runs
   only after >=150s idle.

Device kernel (per core):
  phase A: QT,KT [256,2048] (j-major) and V_aug [2048, 4x65] (ones column per
           head appended -> softmax denominators fall out of the AV matmul).
  phase B: per 512-query tile x 128-key chunk: logitsT = K Q^T via row-packed
           pairs (K=64 each), exp on ACT (no max subtraction; |logit| <~ 50),
           AV accumulation with M=65; then reciprocal + ones-matmul broadcast
           + normalize.
  phase C: out-projection partial [2048, 1024] -> DRAM.
"""

import os
import sys

import numpy as np

for _p in ("/opt/trn_rl_repo",):
    if os.path.isdir(_p) and _p not in sys.path:
        sys.path.insert(0, _p)

from contextlib import ExitStack

import concourse.bass as bass
import concourse.tile as tile
from concourse import bacc, mybir
from concourse._compat import with_exitstack
from concourse.bass_utils import run_bass_kernel_spmd

B, S, D = 2, 2048, 1024
H = 16
HD = 64
NCORES = 8
JG = 256          # channels per core (4 heads)
DC = D // 128     # 8 contraction chunks
QT_TILES = 4      # 4 x 512 query tiles
KC = S // 128     # 16 key chunks
VW = 65           # V columns per head incl. ones column
FP32 = mybir.dt.float32
FP32R = mybir.dt.float32r
FP16 = mybir.dt.float16
BF16 = mybir.dt.bfloat16
EXP = mybir.ActivationFunctionType.Exp


@with_exitstack
def mha_core_kernel(ctx: ExitStack, tc: tile.TileContext,
                    out, xT, wqT, wkT, wvT, bq, bk, bv, owT):
    nc = tc.nc
    # fp32r SBUF tiles hold full fp32 bit patterns; only the PE rounds.
    ctx.enter_context(nc.allow_low_precision("fp32r tiles carry fp32 bits"))

    persist = ctx.enter_context(tc.tile_pool(name="persist", bufs=1))
    QT_sb = persist.tile((128, 2 * S), FP16)
    KT_sb = persist.tile((128, 2 * S), FP16)
    Vaug_sb = persist.tile((128, KC * 4 * VW), BF16)
    attn_outT_sb = persist.tile((128, 2 * S), FP16)
    owT_sb = persist.tile((128, 2 * D), FP16)
    ones2_sb = persist.tile((2, 128), BF16)
    ones1_sb = persist.tile((1, 128), BF16)

    # ---------------- phase A: QKV projections ----------------
    with tc.tile_pool(name="pA", bufs=1) as pA, \
         tc.tile_pool(name="psA", bufs=1, space="PSUM") as psA:
        xT_sb = pA.tile((128, DC * S), FP16)
        wqT_sb = pA.tile((128, DC * JG), FP16)
        wkT_sb = pA.tile((128, DC * JG), FP16)
        wvT_sb = pA.tile((128, DC * JG), FP16)
        bq_sb = pA.tile((128, 2), FP32)
        bk_sb = pA.tile((128, 2), FP32)
        bv_bc = pA.tile((128, JG), FP32)
        ones_f32 = pA.tile((128, 64), FP32)
        ones2_f32 = pA.tile((2, 128), FP32)

        # DMA issue serialization on the sync queue costs ~600ns per
        # dma_start, so batch each weight tensor into ONE 3D-AP descriptor
        # ([dc,128,JG] dram chunks -> contiguous SBUF cols) and each x chunk
        # past dc0 into one [128,S] issue. dc0 stays split by st so the
        # first projection matmul starts as early as possible.
        # DMA issue serialization on the sync queue costs ~600ns per
        # dma_start, so batch each weight tensor into ONE 3D-AP descriptor
        # ([dc,128,JG] dram chunks -> contiguous SBUF cols) and each x chunk
        # past dc0 into one [128,S] issue. dc0 stays split by st so the
        # first projection matmul starts as early as possible. (A host-side
        # pre-shuffle to contiguous [128, DC*JG] DMAs measures ~18us SLOWER
        # than these strided 512B-run descriptors -- don't "fix" it.)
        def chunked_w(src, dst, w):
            # dram [DC*128, w] -> sbuf [128, DC*w] (chunk dc at cols dc*w)
            ap = bass.AP(tensor=src.tensor, offset=src.offset,
                         ap=[[w, 128], [128 * w, DC], [1, w]])
            nc.sync.dma_start(out=dst[:, 0:DC * w], in_=ap)

        # wq's dc0 chunk goes first as its own small issue so the very
        # first matmul only waits ~64KB, not the whole tensor
        nc.sync.dma_start(out=wqT_sb[:, 0:JG], in_=wqT[0:128, :])
        nc.sync.dma_start(
            out=xT_sb[:, 0:512], in_=xT[0:128, 0:512])
        ap = bass.AP(tensor=wqT.tensor, offset=wqT.offset + 128 * JG,
                     ap=[[JG, 128], [128 * JG, DC - 1], [1, JG]])
        nc.sync.dma_start(out=wqT_sb[:, JG:DC * JG], in_=ap)
        for st in range(1, QT_TILES):
            nc.sync.dma_start(
                out=xT_sb[:, st * 512:(st + 1) * 512],
                in_=xT[0:128, st * 512:(st + 1) * 512])
        for dc in range(1, DC):
            nc.sync.dma_start(out=xT_sb[:, dc * S:(dc + 1) * S],
                              in_=xT[dc * 128:(dc + 1) * 128, :])
        chunked_w(wkT, wkT_sb, JG)
        chunked_w(wvT, wvT_sb, JG)
        bq_ap = bass.AP(tensor=bq.tensor, offset=bq.offset,
                        ap=[[1, 128], [128, 2]])
        nc.sync.dma_start(out=bq_sb[:, 0:2], in_=bq_ap)
        bk_ap = bass.AP(tensor=bk.tensor, offset=bk.offset,
                        ap=[[1, 128], [128, 2]])
        nc.sync.dma_start(out=bk_sb[:, 0:2], in_=bk_ap)
        bv_bcast = bass.AP(tensor=bv.tensor, offset=bv.offset,
                           ap=[[0, 128]] + list(bv.ap))
        nc.gpsimd.dma_start(out=bv_bc, in_=bv_bcast)
        ow_ap = bass.AP(tensor=owT.tensor, offset=owT.offset,
                        ap=[[D, 128], [128 * D, 2], [1, D]])
        nc.sync.dma_start(out=owT_sb[:, 0:2 * D], in_=ow_ap)

        # Ones staging AFTER the input-DMA issues: its SBUF->SBUF DMA waits
        # on DVE memsets, and at the head of the sync queue it blocked
        # every weight/x descriptor behind it (~4us of startup).
        # Memset can't emit fp32r (ISA); stage fp32 ones and DVE-copy them
        # into the fp32r tiles (the per-head ones columns of Vaug + the
        # 2-row block-ones used for the packed denominator broadcast).
        nc.vector.memset(ones_f32, 1.0)
        nc.vector.tensor_copy(Vaug_sb[:, HD::VW], ones_f32)
        nc.vector.memset(ones2_f32, 0.0)
        nc.vector.memset(ones2_f32[0:1, 0:64], 1.0)
        # DVE memset can't start at partition 1; DMA-copy the ones block.
        nc.sync.dma_start(out=ones2_f32[1:2, 64:128],
                          in_=ones2_f32[0:1, 0:64])
        nc.vector.tensor_copy(ones2_sb, ones2_f32)
        ones1_f32 = pA.tile((1, 128), FP32)
        nc.vector.memset(ones1_f32, 1.0)
        nc.vector.tensor_copy(ones1_sb, ones1_f32)


        # QT / KT: [j-local, s] as 2 chunks of [128, 2048]. dc-outer so the
        # first matmul only needs the first (wq, x) chunk pair off the wire.
        # (matmul N is ISA-capped at one PSUM bank = 512 fp32.)
        grp = 0
        for w_sb, b_sb, dst in ((wqT_sb, bq_sb, QT_sb), (wkT_sb, bk_sb, KT_sb)):
            for jc in range(2):
                # alternate PSUM tag sets between groups so a group's first
                # matmul never WAR-waits the previous group's bias-add drain
                sset = "AB"[grp % 2]
                grp += 1
                pss = [psA.tile((128, 512), FP32, tag=f"mm{sset}{st}", bufs=1,
                                name=f"mm{sset}{st}") for st in range(QT_TILES)]
                for dc in range(DC - 1):
                    for st in range(QT_TILES):
                        nc.tensor.matmul(
                            pss[st],
                            w_sb[:, dc * JG + jc * 128: dc * JG + (jc + 1) * 128],
                            xT_sb[:, dc * S + st * 512: dc * S + (st + 1) * 512],
                            start=(dc == 0), stop=False,
                        )
                # last dc: bias-add eagerly per st so the next group's WAR
                # on the pss banks clears as early as possible
                dc = DC - 1
                for st in range(QT_TILES):
                    nc.tensor.matmul(
                        pss[st],
                        w_sb[:, dc * JG + jc * 128: dc * JG + (jc + 1) * 128],
                        xT_sb[:, dc * S + st * 512: dc * S + (st + 1) * 512],
                        start=False, stop=True,
                    )
                    nc.vector.tensor_scalar_add(
                        out=dst[:, jc * S + st * 512: jc * S + (st + 1) * 512],
                        in0=pss[st], scalar1=b_sb[:, jc:jc + 1])

        # V: [s, j-local] in 16 chunks, interleaved into Vaug (stride 65).
        # Rotates over three set-A pss banks (their QT/KT WARs are long
        # cleared by now).
        for sc in range(KC):
            psv = psA.tile((128, 512), FP32, tag=f"mmA{sc % 3}", bufs=1,
                           name=f"mmA{sc % 3}")
            ps = psv[:, 0:JG]
            for dc in range(DC):
                nc.tensor.matmul(
                    ps,
                    xT_sb[:, dc * S + sc * 128: dc * S + (sc + 1) * 128],
                    wvT_sb[:, dc * JG:(dc + 1) * JG],
                    start=(dc == 0), stop=(dc == DC - 1),
                )
            base = sc * 4 * VW
            for a in range(4):
                nc.vector.tensor_add(
                    out=Vaug_sb[:, base + a * VW: base + a * VW + HD],
                    in0=ps[:, a * HD:(a + 1) * HD],
                    in1=bv_bc[:, a * HD:(a + 1) * HD])

    # ---------------- phase B + C: attention, interleaved out-proj ----------
    # PSUM banks: av0(2) + av1(2) + lg0(1) + lg1(1) + op(2, shared with the
    # bcast matmul) = 8. lg bufs=1 is free: the next kc's logits matmul only
    # WAR-waits the previous exp, which drains well within the 4-matmul cycle.
    # Out-projection for qt is emitted one half-block later (after qt+1's
    # pair-0 staging), so its attn_outT deps are long satisfied and the PE
    # never stalls on the normalize chain except at the very end.
    with tc.tile_pool(name="pB", bufs=1) as pB, \
         tc.tile_pool(name="psB", bufs=1, space="PSUM") as psB:

        def emit_outproj_group(st, it):
            ps = psB.tile((128, 512), FP32, tag="op", bufs=2, name="op")
            for jc in range(2):
                nc.tensor.matmul(
                    ps,
                    attn_outT_sb[:, jc * S + st * 128:
                                 jc * S + st * 128 + 128],
                    owT_sb[:, jc * D + it * 512: jc * D + (it + 1) * 512],
                    start=(jc == 0), stop=(jc == 1))
            ost = pB.tile((128, 512), FP16, tag="ost", bufs=4, name="ost")
            nc.vector.tensor_copy(ost, ps)
            nc.sync.dma_start(
                out=out[st * 128:(st + 1) * 128,
                        it * 512:(it + 1) * 512],
                in_=ost)

        def emit_outproj_st(st):
            for it in range(2):
                emit_outproj_group(st, it)

        for qt in range(QT_TILES):
            for pair in range(2):
                h0, h1 = 2 * pair, 2 * pair + 1
                av0 = psB.tile((128, 512), FP32, tag="av0", bufs=2, name="av0")
                av1 = psB.tile((128, 512), FP32, tag="av1", bufs=2, name="av1")
                qcol = pair * S + qt * 512
                # kc loop software-pipelined by one stage: AV matmuls for
                # kc-1 issue after the logits matmuls for kc, giving the
                # exps (ACT) a full matmul-pair of slack so the AV matmuls
                # never stall on them.
                pend = None

                def emit_av(kc, ats):
                    for h, at, avp in ((h0, ats[0], av0), (h1, ats[1], av1)):
                        nc.tensor.matmul(
                            avp[0:VW, :],
                            Vaug_sb[:, kc * 4 * VW + h * VW:
                                    kc * 4 * VW + (h + 1) * VW],
                            at,
                            start=(kc == 0), stop=(kc == KC - 1))

                for kc in range(KC):
                    lg0 = psB.tile((128, 512), FP32, tag="lg0", bufs=1,
                                   name="lg0")
                    lg1 = psB.tile((128, 512), FP32, tag="lg1", bufs=1,
                                   name="lg1")
                    kcol = pair * S + kc * 128
                    nc.tensor.matmul(
                        lg0,
                        KT_sb[0:64, kcol:kcol + 128],
                        QT_sb[0:64, qcol:qcol + 512],
                        start=True, stop=True, tile_position=(0, 0))
                    nc.tensor.matmul(
                        lg1,
                        KT_sb[64:128, kcol:kcol + 128],
                        QT_sb[64:128, qcol:qcol + 512],
                        start=True, stop=True, tile_position=(64, 0))
                    if pend is not None:
                        emit_av(*pend)
                    ats = []
                    for h, lg in ((h0, lg0), (h1, lg1)):
                        at = pB.tile((128, 512), BF16, tag=f"at{h % 2}",
                                     bufs=4, name=f"at{h % 2}")
                        nc.scalar.activation(at, lg, EXP)
                        ats.append(at)
                    pend = (kc, ats)
                emit_av(*pend)
                if qt == QT_TILES - 1 and pair == 1:
                    # last block: skip the DMA partition-scatter hop (two K=1
                    # all-ones broadcasts into the now-free av PSUM banks)
                    # and interleave the first out-proj tile with chunked
                    # recip/muls so the tail chain is as short as possible.
                    d0 = pB.tile((1, 512), BF16, tag="dstage", bufs=2,
                                 name="d0")
                    nc.scalar.activation(d0, av0[HD:HD + 1, :],
                                         mybir.ActivationFunctionType.Copy)
                    d1 = pB.tile((1, 512), BF16, tag="drow", bufs=2,
                                 name="d1")
                    nc.vector.tensor_copy(d1, av1[HD:HD + 1, :])
                    bc0 = psB.tile((128, 512), FP32, tag="av0", bufs=2,
                                   name="bc0")
                    nc.tensor.matmul(bc0, ones1_sb, d0, start=True, stop=True)
                    bc1 = psB.tile((128, 512), FP32, tag="av1", bufs=2,
                                   name="bc1")
                    nc.tensor.matmul(bc1, ones1_sb, d1, start=True, stop=True)
                    # reciprocal_approx_fast is only safe on full-tile,
                    # partition-base-0 APs (sub-range/offset variants
                    # mis-evaluate -> NaN), so take the two full-tile
                    # reciprocals up front, then chunk only the cheap muls
                    # so out-proj st12 starts after 128 columns.
                    rcs0 = pB.tile((128, 512), FP32, tag="rcs", bufs=2,
                                   name="rcs0")
                    rcs1 = pB.tile((128, 512), FP32, tag="rcs", bufs=2,
                                   name="rcs1")
                    nc.vector.reciprocal_approx_fast(rcs0, bc0)
                    nc.vector.reciprocal_approx_fast(rcs1, bc1)
                    base = pair * S + qt * 512
                    for lo, hi, sts in ((0, 128, (12,)), (128, 512,
                                                         (13, 14, 15))):
                        nc.vector.tensor_mul(
                            out=attn_outT_sb[0:HD, base + lo:base + hi],
                            in0=av0[0:HD, lo:hi], in1=rcs0[0:HD, lo:hi])
                        nc.vector.tensor_mul(
                            out=attn_outT_sb[HD:128, base + lo:base + hi],
                            in0=av1[0:HD, lo:hi], in1=rcs1[HD:128, lo:hi])
                        for st in sts:
                            emit_outproj_st(st)
                    continue
                # normalize: pack both heads' denominator rows into one
                # [2,512] tile (ACT copies row 0 while DVE copies row 1),
                # broadcast them with a single K=2 block-ones matmul, then
                # one 128-lane reciprocal feeds both heads' muls.
                # The matmul rhs needs the two rows at partition step 1, but
                # DVE/ACT can't write a partition-1 start. Stage both rows in
                # one partition and let a 4KB SBUF->SBUF DMA scatter them.
                # both denominator-row copies on DVE: an ACT copy here makes
                # the next pair's first logits matmul WAR-wait the ACT sem
                # counter past this copy (~0.7us PE stall per pair)
                dstage = pB.tile((1, 1024), BF16, tag="dstage", bufs=2,
                                 name="dstage")
                nc.vector.tensor_copy(dstage[:, 0:512], av0[HD:HD + 1, :])
                nc.vector.tensor_copy(dstage[:, 512:1024], av1[HD:HD + 1, :])
                drows = pB.tile((2, 512), BF16, tag="drow", bufs=2,
                                name="drow")
                nc.sync.dma_start(out=drows, in_=dstage)
                bc = psB.tile((128, 512), FP32, tag="op", bufs=2, name="bc")
                nc.tensor.matmul(bc, ones2_sb, drows, start=True, stop=True)
                rcs = pB.tile((128, 512), FP32, tag="rcs", bufs=2, name="rcs")
                nc.vector.reciprocal_approx_fast(rcs, bc)
                base = pair * S + qt * 512
                nc.vector.tensor_mul(out=attn_outT_sb[0:HD, base:base + 512],
                                     in0=av0[0:HD, :], in1=rcs[0:HD, :])
                nc.vector.tensor_mul(out=attn_outT_sb[HD:128, base:base + 512],
                                     in0=av1[0:HD, :], in1=rcs[HD:128, :])
                if pair == 0 and qt > 0:
                    for st in range(4 * (qt - 1), 4 * (qt - 1) + 4):
                        emit_outproj_st(st)


_NC = None


def _build_nc():
    global _NC
    if _NC is not None:
        return _NC
    nc = bacc.Bacc("TRN2", target_bir_lowering=False, debug=False,
                   num_devices=NCORES)
    xT = nc.dram_tensor("xT", [D, S], FP16, kind="ExternalInput").ap()
    wqT = nc.dram_tensor("wqT", [D, JG], FP16, kind="ExternalInput").ap()
    wkT = nc.dram_tensor("wkT", [D, JG], FP16, kind="ExternalInput").ap()
    wvT = nc.dram_tensor("wvT", [D, JG], FP16, kind="ExternalInput").ap()
    bq = nc.dram_tensor("bq", [JG], FP32, kind="ExternalInput").ap()
    bk = nc.dram_tensor("bk", [JG], FP32, kind="ExternalInput").ap()
    bv = nc.dram_tensor("bv", [JG], FP32, kind="ExternalInput").ap()
    owT = nc.dram_tensor("owT", [JG, D], FP16, kind="ExternalInput").ap()
    out = nc.dram_tensor("out", [S, D], FP16, kind="ExternalOutput").ap()
    with tile.TileContext(nc) as tc:
        mha_core_kernel(tc, out, xT, wqT, wkT, wvT, bq, bk, bv, owT)
    nc.compile()
    _NC = nc
    return nc


def _in_maps(x, kqv_w, kqv_b, out_w):
    maps = []
    xT16 = [np.ascontiguousarray(x[b].T.astype(np.float16)) for b in range(B)]
    for c in range(NCORES):
        b, g = divmod(c, 4)
        sl = slice(g * JG, (g + 1) * JG)
        maps.append({
            "xT": xT16[b],
            "wqT": np.ascontiguousarray(kqv_w[0 * D:1 * D][sl].T.astype(np.float16)),
            "wkT": np.ascontiguousarray(kqv_w[1 * D:2 * D][sl].T.astype(np.float16)),
            "wvT": np.ascontiguousarray(kqv_w[2 * D:3 * D][sl].T.astype(np.float16)),
            "bq": np.ascontiguousarray(kqv_b[0 * D:1 * D][sl]),
            "bk": np.ascontiguousarray(kqv_b[1 * D:2 * D][sl]),
            "bv": np.ascontiguousarray(kqv_b[2 * D:3 * D][sl]),
            "owT": np.ascontiguousarray(out_w[:, sl].T.astype(np.float16)),
        })
    return maps


def run_spmd(x, kqv_w, kqv_b, out_w, out_b, trace=False, tmpdir=None):
    nc = _build_nc()
    res = run_bass_kernel_spmd(nc, _in_maps(x, kqv_w, kqv_b, out_w),
                               list(range(NCORES)), tmpdir=tmpdir, trace=trace)
    parts = [np.asarray(res.results[c]["out"], dtype=np.float32)
             for c in range(NCORES)]
    full = np.stack([
        parts[4 * b] + parts[4 * b + 1] + parts[4 * b + 2] + parts[4 * b + 3]
        + out_b[None, :].astype(np.float32)
        for b in range(B)
    ])
    return full, res


def kernel(**inputs):
    x = np.asarray(inputs["x"], dtype=np.float32)
    kqv_w = np.asarray(inputs["kqv_w"], dtype=np.float32)
    kqv_b = np.asarray(inputs["kqv_b"], dtype=np.float32)
    out_w = np.asarray(inputs["out_w"], dtype=np.float32)
    out_b = np.asarray(inputs["out_b"], dtype=np.float32)
    full, _ = run_spmd(x, kqv_w, kqv_b, out_w, out_b)
    return full



# revision 3
# speedup vs baseline: 1.3267x; 1.3267x over previous
"""MultiHeadAttention (B=2, S=2048, D=1024, H=16) on 8 TRN2 NeuronCores.

Sharding: core c -> batch b = c//4, head-group g = c%4 (4 heads = 256 channels).
Each core computes its 4 heads' attention for its batch plus the partial
out-projection (out_w columns for its channel group); host sums the 4 partials
per batch and adds out_b.

v2 design (from NTFF trace analysis of the v1 baseline, 304-362us):
 - At warm clock (2.4 GHz) the attention inner loop is ACT-bound, not
   PE-bound: each [128,512] exp costs (512+352)/1.2 ns -- a 352-cycle fixed
   overhead per ACTIVATE -- and ACT ran 89% busy in the warm stretch while
   PE matmuls have slack. So:
     * exps are batched: ONE ACTIVATE per key-chunk over a [128,1024] PSUM
       tile (two banks, both heads' logits side by side) -> halves the
       per-instruction overhead on the bottleneck engine.
     * phase A is folded into the attention phase: only KT-jc0, V, and
       QT-jc0-qt0 are computed up front (~22us instead of ~49us serial);
       the remaining Q/K projection matmuls stream into the ACT-bound kc
       loop as PE fill-in (2 matmuls per kc slot), loop is pair-outer so
       jc1 projections are only needed after pair 0 completes.
     * out-projection units fill the pair-1 kc slots the same way.
 - Denominators: ones column per head in Vaug (index HD within each VW=65
   group) makes softmax denominators fall out of the AV matmul; each head's
   denominator row is broadcast with its own K=1 ones-matmul (no SBUF->SBUF
   DMA scatter hop), reciprocal on DVE, per-row tensor_mul normalize.
 - av PSUM banks run bufs=1 (8-bank budget: lg 2x2 + av0 + av1 + op 2x2);
   the av rows are copied to SBUF (fp32, full precision) right after the
   last AV matmul so the WAR for the next block clears early.
 - Dtypes: all matmul operands 2-byte (fp16; exp outputs / denominator rows
   bf16 for range -- logits reach ~50 so e^50 overflows fp16). fp32 PE
   matmuls draw the DEC throttle to its lowest p-state; fp8 measured at
   exactly fp16 speed (duty-based clamp, dtype-blind below 2 bytes).
   Accumulation fp32 in PSUM.
 - Timing is thermally sensitive (~60us swings back-to-back); compare runs
   only after >=150s idle.
"""

import os
import sys

import numpy as np

for _p in ("/opt/trn_rl_repo",):
    if os.path.isdir(_p) and _p not in sys.path:
        sys.path.insert(0, _p)

from collections import deque
from contextlib import ExitStack

import concourse.bass as bass
import concourse.tile as tile
from concourse import bacc, mybir
from concourse._compat import with_exitstack
from concourse.bass_utils import run_bass_kernel_spmd

B, S, D = 2, 2048, 1024
H = 16
HD = 64
NCORES = 8
JG = 256          # channels per core (4 heads)
DC = D // 128     # 8 contraction chunks
QT_TILES = 4      # 4 x 512 query tiles
KC = S // 128     # 16 key chunks
VW = 65           # V columns per head incl. ones column
FP32 = mybir.dt.float32
FP16 = mybir.dt.float16
BF16 = mybir.dt.bfloat16
EXP = mybir.ActivationFunctionType.Exp


@with_exitstack
def mha_core_kernel(ctx: ExitStack, tc: tile.TileContext,
                    out, xT, wqT, wkT, wvT, bq, bk, bv, owT):
    nc = tc.nc
    ctx.enter_context(nc.allow_low_precision("2-byte matmul operands"))

    persist = ctx.enter_context(tc.tile_pool(name="persist", bufs=1))
    QT_sb = persist.tile((128, 2 * S), FP16)
    KT_sb = persist.tile((128, 2 * S), FP16)
    Vaug_sb = persist.tile((128, KC * 4 * VW), BF16)
    attn_outT_sb = persist.tile((128, 2 * S), FP16)
    owT_sb = persist.tile((128, 2 * D), FP16)
    ones1_sb = persist.tile((1, 128), BF16)

    pA = ctx.enter_context(tc.tile_pool(name="pA", bufs=1))
    pB = ctx.enter_context(tc.tile_pool(name="pB", bufs=1))
    ps = ctx.enter_context(tc.tile_pool(name="ps", bufs=1, space="PSUM"))

    xT_sb = pA.tile((128, DC * S), FP16)
    wqT_sb = pA.tile((128, DC * JG), FP16)
    wkT_sb = pA.tile((128, DC * JG), FP16)
    wvT_sb = pA.tile((128, DC * JG), FP16)
    bq_sb = pA.tile((128, 2), FP32)
    bk_sb = pA.tile((128, 2), FP32)
    bv_bc = pA.tile((128, JG), FP32)
    ones_f32 = pA.tile((128, 64), FP32)

    # ---------------- DMA issues (sync queue; ~600ns per dma_start, so
    # batch each weight tensor into ONE 3D-AP strided descriptor; a
    # host-side pre-shuffle to contiguous DMAs measured ~18us SLOWER).
    # KT jc0 runs first on the PE, so wk's dc0 chunk leads.
    def chunked_w(src, dst, lo_dc=0):
        ap = bass.AP(tensor=src.tensor, offset=src.offset + lo_dc * 128 * JG,
                     ap=[[JG, 128], [128 * JG, DC - lo_dc], [1, JG]])
        nc.sync.dma_start(out=dst[:, lo_dc * JG:DC * JG], in_=ap)

    nc.sync.dma_start(out=wkT_sb[:, 0:JG], in_=wkT[0:128, :])
    nc.sync.dma_start(out=xT_sb[:, 0:512], in_=xT[0:128, 0:512])
    chunked_w(wkT, wkT_sb, lo_dc=1)
    for st in range(1, QT_TILES):
        nc.sync.dma_start(
            out=xT_sb[:, st * 512:(st + 1) * 512],
            in_=xT[0:128, st * 512:(st + 1) * 512])
    for dc in range(1, DC):
        nc.sync.dma_start(out=xT_sb[:, dc * S:(dc + 1) * S],
                          in_=xT[dc * 128:(dc + 1) * 128, :])
    chunked_w(wvT, wvT_sb)
    chunked_w(wqT, wqT_sb)
    bq_ap = bass.AP(tensor=bq.tensor, offset=bq.offset,
                    ap=[[1, 128], [128, 2]])
    nc.sync.dma_start(out=bq_sb[:, 0:2], in_=bq_ap)
    bk_ap = bass.AP(tensor=bk.tensor, offset=bk.offset,
                    ap=[[1, 128], [128, 2]])
    nc.sync.dma_start(out=bk_sb[:, 0:2], in_=bk_ap)
    bv_bcast = bass.AP(tensor=bv.tensor, offset=bv.offset,
                       ap=[[0, 128]] + list(bv.ap))
    nc.gpsimd.dma_start(out=bv_bc, in_=bv_bcast)
    ow_ap = bass.AP(tensor=owT.tensor, offset=owT.offset,
                    ap=[[D, 128], [128 * D, 2], [1, D]])
    nc.sync.dma_start(out=owT_sb[:, 0:2 * D], in_=ow_ap)

    # ones: Vaug's per-head denominator columns + the K=1 broadcast row.
    # memset can't emit bf16-from-float cleanly everywhere; stage fp32 and
    # DVE-copy (converts) into the bf16 tiles. No DMA involved.
    nc.vector.memset(ones_f32, 1.0)
    nc.vector.tensor_copy(Vaug_sb[:, HD::VW], ones_f32)
    ones1_f32 = pA.tile((1, 128), FP32)
    nc.vector.memset(ones1_f32, 1.0)
    nc.vector.tensor_copy(ones1_sb, ones1_f32)

    # ---------------- pre-attention projections ----------------
    # KT jc0: dc-outer over 4 query-tile PSUM banks so the first matmul
    # only needs (wk dc0, x dc0-st) off the wire.
    pss = [ps.tile((128, 512), FP32, tag=["lg", "lg", "op", "op"][st],
                   bufs=2, name=f"kt{st}") for st in range(QT_TILES)]
    for dc in range(DC):
        for st in range(QT_TILES):
            nc.tensor.matmul(
                pss[st],
                wkT_sb[:, dc * JG:dc * JG + 128],
                xT_sb[:, dc * S + st * 512:dc * S + (st + 1) * 512],
                start=(dc == 0), stop=(dc == DC - 1),
            )
            if dc == DC - 1:
                nc.vector.tensor_scalar_add(
                    out=KT_sb[:, st * 512:(st + 1) * 512],
                    in0=pss[st], scalar1=bk_sb[:, 0:1])

    # V: [s, j-local] in 16 chunks, interleaved into Vaug (stride 65).
    for sc in range(KC):
        psv = ps.tile((128, 512), FP32, tag=f"av{sc % 2}", bufs=1,
                      name=f"psv{sc % 2}")
        pv = psv[:, 0:JG]
        for dc in range(DC):
            nc.tensor.matmul(
                pv,
                xT_sb[:, dc * S + sc * 128:dc * S + (sc + 1) * 128],
                wvT_sb[:, dc * JG:(dc + 1) * JG],
                start=(dc == 0), stop=(dc == DC - 1),
            )
        base = sc * 4 * VW
        for a in range(4):
            nc.vector.tensor_add(
                out=Vaug_sb[:, base + a * VW:base + a * VW + HD],
                in0=pv[:, a * HD:(a + 1) * HD],
                in1=bv_bc[:, a * HD:(a + 1) * HD])

    # QT jc0 qt0 (dc-inner; everything resident by now)
    def proj_unit(w_sb, b_sb, dst, jc, st):
        """Generator: one (weight, jc, st) projection chunk, 2 matmuls per
        next(), bias-add folded into the last step."""
        pu = ps.tile((128, 512), FP32, tag="op", bufs=2, name="pu")
        for dc in range(DC):
            nc.tensor.matmul(
                pu,
                w_sb[:, dc * JG + jc * 128:dc * JG + (jc + 1) * 128],
                xT_sb[:, dc * S + st * 512:dc * S + (st + 1) * 512],
                start=(dc == 0), stop=(dc == DC - 1),
            )
            if dc % 2 == 1 and dc < DC - 1:
                yield
        nc.vector.tensor_scalar_add(
            out=dst[:, jc * S + st * 512:jc * S + (st + 1) * 512],
            in0=pu, scalar1=b_sb[:, jc:jc + 1])
        yield

    for _ in proj_unit(wqT_sb, bq_sb, QT_sb, 0, 0):
        pass

    # remaining projections stream into the pair-0 kc loop as PE fill-in
    fill_q = deque()
    for st in range(1, QT_TILES):
        fill_q.append(proj_unit(wqT_sb, bq_sb, QT_sb, 0, st))
    for st in range(QT_TILES):
        fill_q.append(proj_unit(wkT_sb, bk_sb, KT_sb, 1, st))
    for st in range(QT_TILES):
        fill_q.append(proj_unit(wqT_sb, bq_sb, QT_sb, 1, st))

    def fill_step():
        while fill_q:
            try:
                next(fill_q[0])
                return
            except StopIteration:
                fill_q.popleft()

    def outproj_unit(st, it):
        """Generator: one [128,512] out-projection tile; 2 matmuls then
        cast+DMA, one next() each."""
        po = ps.tile((128, 512), FP32, tag="op", bufs=2, name="po")
        for jc in range(2):
            nc.tensor.matmul(
                po,
                attn_outT_sb[:, jc * S + st * 128:jc * S + st * 128 + 128],
                owT_sb[:, jc * D + it * 512:jc * D + (it + 1) * 512],
                start=(jc == 0), stop=(jc == 1))
        yield
        ost = pB.tile((128, 512), FP16, tag="ost", bufs=4, name="ost")
        nc.vector.tensor_copy(ost, po)
        nc.sync.dma_start(
            out=out[st * 128:(st + 1) * 128, it * 512:(it + 1) * 512],
            in_=ost)
        yield

    def queue_outproj(qt):
        for st in range(4 * qt, 4 * qt + 4):
            for it in range(2):
                fill_q.append(outproj_unit(st, it))

    # ---------------- attention (pair-outer) ----------------
    for pair in range(2):
        for qt in range(QT_TILES):
            if pair == 1 and qt > 0:
                queue_outproj(qt - 1)
            av0 = ps.tile((128, 512), FP32, tag="av0", bufs=1, name="av0")
            av1 = ps.tile((128, 512), FP32, tag="av1", bufs=1, name="av1")
            qcol = pair * S + qt * 512
            pend = None

            def emit_av(kc, at):
                for h, avp, off in ((2 * pair, av0, 0),
                                    (2 * pair + 1, av1, 512)):
                    nc.tensor.matmul(
                        avp[0:VW, :],
                        Vaug_sb[:, kc * 4 * VW + h * VW:
                                kc * 4 * VW + (h + 1) * VW],
                        at[:, off:off + 512],
                        start=(kc == 0), stop=(kc == KC - 1))

            # kc loop, software-pipelined one stage: fill-in matmuls go
            # between the logits matmuls (which never stall) and the AV
            # matmuls for kc-1 (which wait on the exp), so the in-order PE
            # queue does fill work during the ACT dependency window.
            for kc in range(KC):
                lg = ps.tile((128, 1024), FP32, tag="lg", bufs=2, name="lg")
                kcol = pair * S + kc * 128
                nc.tensor.matmul(
                    lg[:, 0:512],
                    KT_sb[0:64, kcol:kcol + 128],
                    QT_sb[0:64, qcol:qcol + 512],
                    start=True, stop=True, tile_position=(0, 0))
                nc.tensor.matmul(
                    lg[:, 512:1024],
                    KT_sb[64:128, kcol:kcol + 128],
                    QT_sb[64:128, qcol:qcol + 512],
                    start=True, stop=True, tile_position=(64, 0))
                fill_step()
                if pend is not None:
                    emit_av(*pend)
                at = pB.tile((128, 1024), BF16, tag="at", bufs=3, name="at")
                nc.scalar.activation(at, lg, EXP)
                pend = (kc, at)
            emit_av(*pend)

            # normalize: copy av rows to SBUF fp32 (frees the av banks for
            # the next block), broadcast each head's denominator row with a
            # K=1 ones-matmul, reciprocal, per-row mul into attn_outT.
            avs0 = pB.tile((VW, 512), FP32, tag="avs0", bufs=2, name="avs0")
            nc.vector.tensor_copy(avs0, av0[0:VW, :])
            avs1 = pB.tile((VW, 512), FP32, tag="avs1", bufs=2, name="avs1")
            nc.vector.tensor_copy(avs1, av1[0:VW, :])
            d0 = pB.tile((1, 512), BF16, tag="d0", bufs=2, name="d0")
            nc.vector.tensor_copy(d0, av0[HD:HD + 1, :])
            d1 = pB.tile((1, 512), BF16, tag="d1", bufs=2, name="d1")
            nc.vector.tensor_copy(d1, av1[HD:HD + 1, :])
            bc0 = ps.tile((128, 512), FP32, tag="op", bufs=2, name="bc0")
            nc.tensor.matmul(bc0, ones1_sb, d0, start=True, stop=True)
            bc1 = ps.tile((128, 512), FP32, tag="op", bufs=2, name="bc1")
            nc.tensor.matmul(bc1, ones1_sb, d1, start=True, stop=True)
            # reciprocal_approx_fast is only safe on full-tile,
            # partition-base-0 APs (sub-range variants mis-evaluate -> NaN)
            rcs0 = pB.tile((128, 512), FP32, tag="rcs", bufs=2, name="rcs0")
            nc.vector.reciprocal_approx_fast(rcs0, bc0)
            rcs1 = pB.tile((128, 512), FP32, tag="rcs", bufs=2, name="rcs1")
            nc.vector.reciprocal_approx_fast(rcs1, bc1)
            base = pair * S + qt * 512
            nc.vector.tensor_mul(out=attn_outT_sb[0:HD, base:base + 512],
                                 in0=avs0[0:HD, :], in1=rcs0[0:HD, :])
            nc.vector.tensor_mul(out=attn_outT_sb[HD:128, base:base + 512],
                                 in0=avs1[0:HD, :], in1=rcs1[0:HD, :])

    # tail: last query tile's out-projection
    queue_outproj(QT_TILES - 1)
    while fill_q:
        fill_step()


_NC = None


def _build_nc():
    global _NC
    if _NC is not None:
        return _NC
    nc = bacc.Bacc("TRN2", target_bir_lowering=False, debug=False,
                   num_devices=NCORES)
    xT = nc.dram_tensor("xT", [D, S], FP16, kind="ExternalInput").ap()
    wqT = nc.dram_tensor("wqT", [D, JG], FP16, kind="ExternalInput").ap()
    wkT = nc.dram_tensor("wkT", [D, JG], FP16, kind="ExternalInput").ap()
    wvT = nc.dram_tensor("wvT", [D, JG], FP16, kind="ExternalInput").ap()
    bq = nc.dram_tensor("bq", [JG], FP32, kind="ExternalInput").ap()
    bk = nc.dram_tensor("bk", [JG], FP32, kind="ExternalInput").ap()
    bv = nc.dram_tensor("bv", [JG], FP32, kind="ExternalInput").ap()
    owT = nc.dram_tensor("owT", [JG, D], FP16, kind="ExternalInput").ap()
    out = nc.dram_tensor("out", [S, D], FP16, kind="ExternalOutput").ap()
    with tile.TileContext(nc) as tc:
        mha_core_kernel(tc, out, xT, wqT, wkT, wvT, bq, bk, bv, owT)
    nc.compile()
    _NC = nc
    return nc


def _in_maps(x, kqv_w, kqv_b, out_w):
    maps = []
    xT16 = [np.ascontiguousarray(x[b].T.astype(np.float16)) for b in range(B)]
    for c in range(NCORES):
        b, g = divmod(c, 4)
        sl = slice(g * JG, (g + 1) * JG)
        maps.append({
            "xT": xT16[b],
            "wqT": np.ascontiguousarray(kqv_w[0 * D:1 * D][sl].T.astype(np.float16)),
            "wkT": np.ascontiguousarray(kqv_w[1 * D:2 * D][sl].T.astype(np.float16)),
            "wvT": np.ascontiguousarray(kqv_w[2 * D:3 * D][sl].T.astype(np.float16)),
            "bq": np.ascontiguousarray(kqv_b[0 * D:1 * D][sl]),
            "bk": np.ascontiguousarray(kqv_b[1 * D:2 * D][sl]),
            "bv": np.ascontiguousarray(kqv_b[2 * D:3 * D][sl]),
            "owT": np.ascontiguousarray(out_w[:, sl].T.astype(np.float16)),
        })
    return maps


def run_spmd(x, kqv_w, kqv_b, out_w, out_b, trace=False, tmpdir=None):
    nc = _build_nc()
    res = run_bass_kernel_spmd(nc, _in_maps(x, kqv_w, kqv_b, out_w),
                               list(range(NCORES)), tmpdir=tmpdir, trace=trace)
    parts = [np.asarray(res.results[c]["out"], dtype=np.float32)
             for c in range(NCORES)]
    full = np.stack([
        parts[4 * b] + parts[4 * b + 1] + parts[4 * b + 2] + parts[4 * b + 3]
        + out_b[None, :].astype(np.float32)
        for b in range(B)
    ])
    return full, res


def kernel(**inputs):
    x = np.asarray(inputs["x"], dtype=np.float32)
    kqv_w = np.asarray(inputs["kqv_w"], dtype=np.float32)
    kqv_b = np.asarray(inputs["kqv_b"], dtype=np.float32)
    out_w = np.asarray(inputs["out_w"], dtype=np.float32)
    out_b = np.asarray(inputs["out_b"], dtype=np.float32)
    full, _ = run_spmd(x, kqv_w, kqv_b, out_w, out_b)
    return full


# revision 7
# speedup vs baseline: 1.3971x; 1.0530x over previous
"""MultiHeadAttention (B=2, S=2048, D=1024, H=16) on 8 TRN2 NeuronCores.

Sharding: core c -> batch b = c//4, head-group g = c%4 (4 heads = 256 channels).
Each core computes its 4 heads' attention for its batch plus the partial
out-projection (out_w columns for its channel group); host sums the 4 partials
per batch and adds out_b.

v2 design (from NTFF trace analysis of the v1 baseline, 304-362us):
 - At warm clock (2.4 GHz) the attention inner loop is ACT-bound, not
   PE-bound: each [128,512] exp costs (512+352)/1.2 ns -- a 352-cycle fixed
   overhead per ACTIVATE -- and ACT ran 89% busy in the warm stretch while
   PE matmuls have slack. So:
     * exps are batched: ONE ACTIVATE per key-chunk over a [128,1024] PSUM
       tile (two banks, both heads' logits side by side) -> halves the
       per-instruction overhead on the bottleneck engine.
     * phase A is folded into the attention phase: only KT-jc0, V, and
       QT-jc0-qt0 are computed up front (~22us instead of ~49us serial);
       the remaining Q/K projection matmuls stream into the ACT-bound kc
       loop as PE fill-in (2 matmuls per kc slot), loop is pair-outer so
       jc1 projections are only needed after pair 0 completes.
     * out-projection units fill the pair-1 kc slots the same way.
 - Denominators: ones column per head in Vaug (index HD within each VW=65
   group) makes softmax denominators fall out of the AV matmul; each head's
   denominator row is broadcast with its own K=1 ones-matmul (no SBUF->SBUF
   DMA scatter hop), reciprocal on DVE, per-row tensor_mul normalize.
 - av PSUM banks run bufs=1 (8-bank budget: lg 2x2 + av0 + av1 + op 2x2);
   the av rows are copied to SBUF (fp32, full precision) right after the
   last AV matmul so the WAR for the next block clears early.
 - Dtypes: all matmul operands 2-byte (fp16; exp outputs / denominator rows
   bf16 for range -- logits reach ~50 so e^50 overflows fp16). fp32 PE
   matmuls draw the DEC throttle to its lowest p-state; fp8 measured at
   exactly fp16 speed (duty-based clamp, dtype-blind below 2 bytes).
   Accumulation fp32 in PSUM.
 - Timing is thermally sensitive (~60us swings back-to-back); compare runs
   only after >=150s idle.
"""

import os
import sys

import numpy as np

for _p in ("/opt/trn_rl_repo",):
    if os.path.isdir(_p) and _p not in sys.path:
        sys.path.insert(0, _p)

from collections import deque
from contextlib import ExitStack

import concourse.bass as bass
import concourse.tile as tile
from concourse import bacc, mybir
from concourse._compat import with_exitstack
from concourse.bass_utils import run_bass_kernel_spmd

B, S, D = 2, 2048, 1024
H = 16
HD = 64
NCORES = 8
JG = 256          # channels per core (4 heads)
DC = D // 128     # 8 contraction chunks
QT_TILES = 4      # 4 x 512 query tiles
KC = S // 128     # 16 key chunks
VW = 65           # V columns per head incl. ones column
FP32 = mybir.dt.float32
FP16 = mybir.dt.float16
BF16 = mybir.dt.bfloat16
EXP = mybir.ActivationFunctionType.Exp


@with_exitstack
def mha_core_kernel(ctx: ExitStack, tc: tile.TileContext,
                    out, xT, wqT, wkT, wvT, bq, bk, bv, owT):
    nc = tc.nc
    ctx.enter_context(nc.allow_low_precision("2-byte matmul operands"))

    persist = ctx.enter_context(tc.tile_pool(name="persist", bufs=1))
    QT_sb = persist.tile((128, 2 * S), FP16)
    KT_sb = persist.tile((128, 2 * S), FP16)
    Vaug_sb = persist.tile((128, KC * 4 * VW), BF16)
    attn_outT_sb = persist.tile((128, 2 * S), FP16)
    owT_sb = persist.tile((128, 2 * D), FP16)
    ones1_sb = persist.tile((1, 128), BF16)

    pA = ctx.enter_context(tc.tile_pool(name="pA", bufs=1))
    pB = ctx.enter_context(tc.tile_pool(name="pB", bufs=1))
    ps = ctx.enter_context(tc.tile_pool(name="ps", bufs=1, space="PSUM"))

    xT_sb = pA.tile((128, DC * S), FP16)
    wqT_sb = pA.tile((128, DC * JG), FP16)
    wkT_sb = pA.tile((128, DC * JG), FP16)
    wvT_sb = pA.tile((128, DC * JG), FP16)
    bq_sb = pA.tile((128, 2), FP32)
    bk_sb = pA.tile((128, 2), FP32)
    bv_bc = pA.tile((128, JG), FP32)
    ones_f32 = pA.tile((128, 64), FP32)

    # ---------------- DMA issues (sync queue; ~600ns per dma_start, so
    # batch each weight tensor into ONE 3D-AP strided descriptor; a
    # host-side pre-shuffle to contiguous DMAs measured ~18us SLOWER).
    # KT jc0 runs first on the PE, so wk's dc0 chunk leads.
    def chunked_w(src, dst, lo_dc=0):
        ap = bass.AP(tensor=src.tensor, offset=src.offset + lo_dc * 128 * JG,
                     ap=[[JG, 128], [128 * JG, DC - lo_dc], [1, JG]])
        nc.sync.dma_start(out=dst[:, lo_dc * JG:DC * JG], in_=ap)

    nc.sync.dma_start(out=wkT_sb[:, 0:JG], in_=wkT[0:128, :])
    nc.sync.dma_start(out=xT_sb[:, 0:512], in_=xT[0:128, 0:512])
    chunked_w(wkT, wkT_sb, lo_dc=1)
    for st in range(1, QT_TILES):
        nc.sync.dma_start(
            out=xT_sb[:, st * 512:(st + 1) * 512],
            in_=xT[0:128, st * 512:(st + 1) * 512])
    for dc in range(1, DC):
        nc.sync.dma_start(out=xT_sb[:, dc * S:(dc + 1) * S],
                          in_=xT[dc * 128:(dc + 1) * 128, :])
    chunked_w(wvT, wvT_sb)
    chunked_w(wqT, wqT_sb)
    bq_ap = bass.AP(tensor=bq.tensor, offset=bq.offset,
                    ap=[[1, 128], [128, 2]])
    nc.sync.dma_start(out=bq_sb[:, 0:2], in_=bq_ap)
    bk_ap = bass.AP(tensor=bk.tensor, offset=bk.offset,
                    ap=[[1, 128], [128, 2]])
    nc.sync.dma_start(out=bk_sb[:, 0:2], in_=bk_ap)
    bv_bcast = bass.AP(tensor=bv.tensor, offset=bv.offset,
                       ap=[[0, 128]] + list(bv.ap))
    nc.gpsimd.dma_start(out=bv_bc, in_=bv_bcast)
    ow_ap = bass.AP(tensor=owT.tensor, offset=owT.offset,
                    ap=[[D, 128], [128 * D, 2], [1, D]])
    nc.sync.dma_start(out=owT_sb[:, 0:2 * D], in_=ow_ap)

    # ones: Vaug's per-head denominator columns + the K=1 broadcast row.
    # memset can't emit bf16-from-float cleanly everywhere; stage fp32 and
    # DVE-copy (converts) into the bf16 tiles. No DMA involved.
    nc.vector.memset(ones_f32, 1.0)
    nc.vector.tensor_copy(Vaug_sb[:, HD::VW], ones_f32)
    ones1_f32 = pA.tile((1, 128), FP32)
    nc.vector.memset(ones1_f32, 1.0)
    nc.vector.tensor_copy(ones1_sb, ones1_f32)

    # ---------------- pre-attention projections ----------------
    # KT jc0: dc-outer over 4 query-tile PSUM banks so the first matmul
    # only needs (wk dc0, x dc0-st) off the wire.
    pss = [ps.tile((128, 512), FP32, tag=["lg", "lg", "op", "op"][st],
                   bufs=2, name=f"kt{st}") for st in range(QT_TILES)]
    for dc in range(DC):
        for st in range(QT_TILES):
            nc.tensor.matmul(
                pss[st],
                wkT_sb[:, dc * JG:dc * JG + 128],
                xT_sb[:, dc * S + st * 512:dc * S + (st + 1) * 512],
                start=(dc == 0), stop=(dc == DC - 1),
            )
            if dc == DC - 1:
                nc.vector.tensor_scalar_add(
                    out=KT_sb[:, st * 512:(st + 1) * 512],
                    in0=pss[st], scalar1=bk_sb[:, 0:1])

    def proj_unit(w_sb, b_sb, dst, jc, st):
        """Generator: one (weight, jc, st) projection chunk, 2 matmuls per
        next(), bias-add folded into the last step."""
        pu = ps.tile((128, 512), FP32, tag="op", bufs=2, name="pu")
        for dc in range(DC):
            nc.tensor.matmul(
                pu,
                w_sb[:, dc * JG + jc * 128:dc * JG + (jc + 1) * 128],
                xT_sb[:, dc * S + st * 512:dc * S + (st + 1) * 512],
                start=(dc == 0), stop=(dc == DC - 1),
            )
            if dc % 2 == 1 and dc < DC - 1:
                yield
        nc.vector.tensor_scalar_add(
            out=dst[:, jc * S + st * 512:jc * S + (st + 1) * 512],
            in0=pu, scalar1=b_sb[:, jc:jc + 1])
        yield

    for _ in proj_unit(wqT_sb, bq_sb, QT_sb, 0, 0):
        pass

    # V: [s, j-local] in 16 chunks, interleaved into Vaug (stride 65).
    # Emitted AFTER QT-qt0 so the list scheduler can start attention as
    # soon as (KT jc0, QT qt0) are done; V fills PE slack during the
    # ACT-bound early blocks. 4-deep PSUM rotation + a single strided
    # bias-add per chunk keep V from being gated by its own WAR chain
    # (2-slot rotation + 4 small adds measured as the pre-attention drag).
    def ap3(t, groups, width):
        return bass.AP(tensor=t.tensor, offset=t.offset,
                       ap=[list(t.ap[0]), [VW, groups], [1, width]])

    for sc in range(KC):
        psv = ps.tile((128, 512), FP32,
                      tag=["av0", "av1", "op", "op"][sc % 4],
                      bufs=1 if sc % 4 < 2 else 2, name=f"psv{sc % 4}")
        pv = psv[:, 0:JG]
        for dc in range(DC):
            nc.tensor.matmul(
                pv,
                xT_sb[:, dc * S + sc * 128:dc * S + (sc + 1) * 128],
                wvT_sb[:, dc * JG:(dc + 1) * JG],
                start=(dc == 0), stop=(dc == DC - 1),
            )
        base = sc * 4 * VW
        pv4 = bass.AP(tensor=pv.tensor, offset=pv.offset,
                      ap=[list(pv.ap[0]), [HD, 4], [1, HD]])
        bv4 = bass.AP(tensor=bv_bc.tensor, offset=bv_bc.offset,
                      ap=[list(bv_bc.ap[0]), [HD, 4], [1, HD]])
        nc.vector.tensor_add(
            out=ap3(Vaug_sb[:, base:base + 4 * VW], 4, HD),
            in0=pv4, in1=bv4)

    # remaining projections stream into the pair-0 kc loop as PE fill-in
    fill_q = deque()
    for st in range(1, QT_TILES):
        fill_q.append(proj_unit(wqT_sb, bq_sb, QT_sb, 0, st))
    for st in range(QT_TILES):
        fill_q.append(proj_unit(wkT_sb, bk_sb, KT_sb, 1, st))
    for st in range(QT_TILES):
        fill_q.append(proj_unit(wqT_sb, bq_sb, QT_sb, 1, st))

    def fill_step():
        while fill_q:
            try:
                next(fill_q[0])
                return
            except StopIteration:
                fill_q.popleft()

    def outproj_unit(st, it):
        """Generator: one [128,512] out-projection tile; 2 matmuls then
        cast+DMA, one next() each."""
        po = ps.tile((128, 512), FP32, tag="op", bufs=2, name="po")
        for jc in range(2):
            nc.tensor.matmul(
                po,
                attn_outT_sb[:, jc * S + st * 128:jc * S + st * 128 + 128],
                owT_sb[:, jc * D + it * 512:jc * D + (it + 1) * 512],
                start=(jc == 0), stop=(jc == 1))
        yield
        ost = pB.tile((128, 512), FP16, tag="ost", bufs=4, name="ost")
        nc.vector.tensor_copy(ost, po)
        nc.sync.dma_start(
            out=out[st * 128:(st + 1) * 128, it * 512:(it + 1) * 512],
            in_=ost)
        yield

    def queue_outproj(qt):
        for st in range(4 * qt, 4 * qt + 4):
            for it in range(2):
                fill_q.append(outproj_unit(st, it))

    # ---------------- attention (pair-outer) ----------------
    # Each block's normalize (bc matmuls + recip + muls) is DEFERRED into
    # the next block's first kc slot: emitting it at block end puts the bc
    # matmuls (which wait a ~1.5us DVE copy chain) ahead of the next
    # block's logits in the PE stream and stalls ACT ~2.5us per boundary.
    def make_normalize(avs0, avs1, d0, d1, base, chunked=False):
        def emit():
            bc0 = ps.tile((128, 512), FP32, tag="op", bufs=2, name="bc0")
            nc.tensor.matmul(bc0, ones1_sb, d0, start=True, stop=True)
            bc1 = ps.tile((128, 512), FP32, tag="op", bufs=2, name="bc1")
            nc.tensor.matmul(bc1, ones1_sb, d1, start=True, stop=True)
            # reciprocal_approx_fast is only safe on full-tile,
            # partition-base-0 APs (sub-ranges mis-evaluate -> NaN)
            rcs0 = pB.tile((128, 512), FP32, tag="rcs", bufs=2, name="rcs0")
            nc.vector.reciprocal_approx_fast(rcs0, bc0)
            rcs1 = pB.tile((128, 512), FP32, tag="rcs", bufs=2, name="rcs1")
            nc.vector.reciprocal_approx_fast(rcs1, bc1)
            chunks = ((0, 128, (12,)), (128, 512, (13, 14, 15))) if chunked \
                else ((0, 512, ()),)
            for lo, hi, sts in chunks:
                nc.vector.tensor_mul(
                    out=attn_outT_sb[0:HD, base + lo:base + hi],
                    in0=avs0[0:HD, lo:hi], in1=rcs0[0:HD, lo:hi])
                nc.vector.tensor_mul(
                    out=attn_outT_sb[HD:128, base + lo:base + hi],
                    in0=avs1[0:HD, lo:hi], in1=rcs1[0:HD, lo:hi])
                for st in sts:
                    for it in range(2):
                        for _ in outproj_unit(st, it):
                            pass
        return emit

    norm_pend = None
    for pair in range(2):
        for qt in range(QT_TILES):
            av0 = ps.tile((128, 512), FP32, tag="av0", bufs=1, name="av0")
            av1 = ps.tile((128, 512), FP32, tag="av1", bufs=1, name="av1")
            qcol = pair * S + qt * 512
            pend = None

            def emit_av(kc, at):
                for h, avp, off in ((2 * pair, av0, 0),
                                    (2 * pair + 1, av1, 512)):
                    nc.tensor.matmul(
                        avp[0:VW, :],
                        Vaug_sb[:, kc * 4 * VW + h * VW:
                                kc * 4 * VW + (h + 1) * VW],
                        at[:, off:off + 512],
                        start=(kc == 0), stop=(kc == KC - 1))

            # kc loop, software-pipelined one stage: fill-in matmuls go
            # between the logits matmuls (which never stall) and the AV
            # matmuls for kc-1 (which wait on the exp).
            for kc in range(KC):
                lg = ps.tile((128, 1024), FP32, tag="lg", bufs=2, name="lg")
                kcol = pair * S + kc * 128
                nc.tensor.matmul(
                    lg[:, 0:512],
                    KT_sb[0:64, kcol:kcol + 128],
                    QT_sb[0:64, qcol:qcol + 512],
                    start=True, stop=True, tile_position=(0, 0))
                nc.tensor.matmul(
                    lg[:, 512:1024],
                    KT_sb[64:128, kcol:kcol + 128],
                    QT_sb[64:128, qcol:qcol + 512],
                    start=True, stop=True, tile_position=(64, 0))
                fill_step()
                if pend is not None:
                    emit_av(*pend)
                at = pB.tile((128, 1024), BF16, tag="at", bufs=3, name="at")
                nc.scalar.activation(at, lg, EXP)
                pend = (kc, at)
                if kc == 0:
                    if norm_pend is not None:
                        norm_pend()
                        norm_pend = None
                    if pair == 1 and qt > 0:
                        queue_outproj(qt - 1)
            emit_av(*pend)

            # denominator rows first (the bc matmuls need them soonest),
            # then the av rows to SBUF fp32 -- frees the av banks so the
            # next block's first AV matmul doesn't WAR-wait the normalize.
            d0 = pB.tile((1, 512), BF16, tag="d0", bufs=2, name="d0")
            nc.vector.tensor_copy(d0, av0[HD:HD + 1, :])
            d1 = pB.tile((1, 512), BF16, tag="d1", bufs=2, name="d1")
            nc.vector.tensor_copy(d1, av1[HD:HD + 1, :])
            avs0 = pB.tile((VW, 512), FP32, tag="avs0", bufs=2, name="avs0")
            nc.vector.tensor_copy(avs0, av0[0:VW, :])
            avs1 = pB.tile((VW, 512), FP32, tag="avs1", bufs=2, name="avs1")
            nc.vector.tensor_copy(avs1, av1[0:VW, :])
            base = pair * S + qt * 512
            norm_pend = make_normalize(
                avs0, avs1, d0, d1, base,
                chunked=(pair == 1 and qt == QT_TILES - 1))

    # tail: drain leftover fill units, then the last block's normalize with
    # its out-projection st-units interleaved
    while fill_q:
        fill_step()
    norm_pend()


_NC = None


def _build_nc():
    global _NC
    if _NC is not None:
        return _NC
    nc = bacc.Bacc("TRN2", target_bir_lowering=False, debug=False,
                   num_devices=NCORES)
    xT = nc.dram_tensor("xT", [D, S], FP16, kind="ExternalInput").ap()
    wqT = nc.dram_tensor("wqT", [D, JG], FP16, kind="ExternalInput").ap()
    wkT = nc.dram_tensor("wkT", [D, JG], FP16, kind="ExternalInput").ap()
    wvT = nc.dram_tensor("wvT", [D, JG], FP16, kind="ExternalInput").ap()
    bq = nc.dram_tensor("bq", [JG], FP32, kind="ExternalInput").ap()
    bk = nc.dram_tensor("bk", [JG], FP32, kind="ExternalInput").ap()
    bv = nc.dram_tensor("bv", [JG], FP32, kind="ExternalInput").ap()
    owT = nc.dram_tensor("owT", [JG, D], FP16, kind="ExternalInput").ap()
    out = nc.dram_tensor("out", [S, D], FP16, kind="ExternalOutput").ap()
    with tile.TileContext(nc) as tc:
        mha_core_kernel(tc, out, xT, wqT, wkT, wvT, bq, bk, bv, owT)
    nc.compile()
    _NC = nc
    return nc


def _in_maps(x, kqv_w, kqv_b, out_w):
    maps = []
    xT16 = [np.ascontiguousarray(x[b].T.astype(np.float16)) for b in range(B)]
    for c in range(NCORES):
        b, g = divmod(c, 4)
        sl = slice(g * JG, (g + 1) * JG)
        maps.append({
            "xT": xT16[b],
            "wqT": np.ascontiguousarray(kqv_w[0 * D:1 * D][sl].T.astype(np.float16)),
            "wkT": np.ascontiguousarray(kqv_w[1 * D:2 * D][sl].T.astype(np.float16)),
            "wvT": np.ascontiguousarray(kqv_w[2 * D:3 * D][sl].T.astype(np.float16)),
            "bq": np.ascontiguousarray(kqv_b[0 * D:1 * D][sl]),
            "bk": np.ascontiguousarray(kqv_b[1 * D:2 * D][sl]),
            "bv": np.ascontiguousarray(kqv_b[2 * D:3 * D][sl]),
            "owT": np.ascontiguousarray(out_w[:, sl].T.astype(np.float16)),
        })
    return maps


def run_spmd(x, kqv_w, kqv_b, out_w, out_b, trace=False, tmpdir=None):
    nc = _build_nc()
    res = run_bass_kernel_spmd(nc, _in_maps(x, kqv_w, kqv_b, out_w),
                               list(range(NCORES)), tmpdir=tmpdir, trace=trace)
    parts = [np.asarray(res.results[c]["out"], dtype=np.float32)
             for c in range(NCORES)]
    full = np.stack([
        parts[4 * b] + parts[4 * b + 1] + parts[4 * b + 2] + parts[4 * b + 3]
        + out_b[None, :].astype(np.float32)
        for b in range(B)
    ])
    return full, res


def kernel(**inputs):
    x = np.asarray(inputs["x"], dtype=np.float32)
    kqv_w = np.asarray(inputs["kqv_w"], dtype=np.float32)
    kqv_b = np.asarray(inputs["kqv_b"], dtype=np.float32)
    out_w = np.asarray(inputs["out_w"], dtype=np.float32)
    out_b = np.asarray(inputs["out_b"], dtype=np.float32)
    full, _ = run_spmd(x, kqv_w, kqv_b, out_w, out_b)
    return full


# revision 11
# speedup vs baseline: 1.3991x; 1.0014x over previous
"""MultiHeadAttention (B=2, S=2048, D=1024, H=16) on 8 TRN2 NeuronCores.

Sharding: core c -> batch b = c//4, head-group g = c%4 (4 heads = 256 channels).
Each core computes its 4 heads' attention for its batch plus the partial
out-projection (out_w columns for its channel group); host sums the 4 partials
per batch and adds out_b.

v2 design (from NTFF trace analysis of the v1 baseline, 304-362us):
 - At warm clock (2.4 GHz) the attention inner loop is ACT-bound, not
   PE-bound: each [128,512] exp costs (512+352)/1.2 ns -- a 352-cycle fixed
   overhead per ACTIVATE -- and ACT ran 89% busy in the warm stretch while
   PE matmuls have slack. So:
     * exps are batched: ONE ACTIVATE per key-chunk over a [128,1024] PSUM
       tile (two banks, both heads' logits side by side) -> halves the
       per-instruction overhead on the bottleneck engine.
     * phase A is folded into the attention phase: only KT-jc0, V, and
       QT-jc0-qt0 are computed up front (~22us instead of ~49us serial);
       the remaining Q/K projection matmuls stream into the ACT-bound kc
       loop as PE fill-in (2 matmuls per kc slot), loop is pair-outer so
       jc1 projections are only needed after pair 0 completes.
     * out-projection units fill the pair-1 kc slots the same way.
 - Denominators: ones column per head in Vaug (index HD within each VW=65
   group) makes softmax denominators fall out of the AV matmul; each head's
   denominator row is broadcast with its own K=1 ones-matmul (no SBUF->SBUF
   DMA scatter hop), reciprocal on DVE, per-row tensor_mul normalize.
 - av PSUM banks run bufs=1 (8-bank budget: lg 2x2 + av0 + av1 + op 2x2);
   the av rows are copied to SBUF (fp32, full precision) right after the
   last AV matmul so the WAR for the next block clears early.
 - Dtypes: all matmul operands 2-byte (fp16; exp outputs / denominator rows
   bf16 for range -- logits reach ~50 so e^50 overflows fp16). fp32 PE
   matmuls draw the DEC throttle to its lowest p-state; fp8 measured at
   exactly fp16 speed (duty-based clamp, dtype-blind below 2 bytes).
   Accumulation fp32 in PSUM.
 - Timing is thermally sensitive (~60us swings back-to-back); compare runs
   only after >=150s idle.
"""

import os
import sys

import numpy as np

for _p in ("/opt/trn_rl_repo",):
    if os.path.isdir(_p) and _p not in sys.path:
        sys.path.insert(0, _p)

from collections import deque
from contextlib import ExitStack

import concourse.bass as bass
import concourse.tile as tile
from concourse import bacc, mybir
from concourse._compat import with_exitstack
from concourse.bass_utils import run_bass_kernel_spmd

B, S, D = 2, 2048, 1024
H = 16
HD = 64
NCORES = 8
JG = 256          # channels per core (4 heads)
DC = D // 128     # 8 contraction chunks
QT_TILES = 4      # 4 x 512 query tiles
KC = S // 128     # 16 key chunks
VW = 65           # V columns per head incl. ones column
FP32 = mybir.dt.float32
FP16 = mybir.dt.float16
BF16 = mybir.dt.bfloat16
EXP = mybir.ActivationFunctionType.Exp


@with_exitstack
def mha_core_kernel(ctx: ExitStack, tc: tile.TileContext,
                    out, xT, wqT, wkT, wvT, bq, bk, bv, owT):
    nc = tc.nc
    ctx.enter_context(nc.allow_low_precision("2-byte matmul operands"))

    persist = ctx.enter_context(tc.tile_pool(name="persist", bufs=1))
    QT_sb = persist.tile((128, 2 * S), FP16)
    KT_sb = persist.tile((128, 2 * S), FP16)
    Vaug_sb = persist.tile((128, KC * 4 * VW), BF16)
    attn_outT_sb = persist.tile((128, 2 * S), FP16)
    owT_sb = persist.tile((128, 2 * D), FP16)
    ones1_sb = persist.tile((1, 128), BF16)

    pA = ctx.enter_context(tc.tile_pool(name="pA", bufs=1))
    pB = ctx.enter_context(tc.tile_pool(name="pB", bufs=1))
    ps = ctx.enter_context(tc.tile_pool(name="ps", bufs=1, space="PSUM"))

    xT_sb = pA.tile((128, DC * S), FP16)
    wqT_sb = pA.tile((128, DC * JG), FP16)
    wkT_sb = pA.tile((128, DC * JG), FP16)
    wvT_sb = pA.tile((128, DC * JG), FP16)
    bq_sb = pA.tile((128, 2), FP32)
    bk_sb = pA.tile((128, 2), FP32)
    bv_bc = pA.tile((128, JG), FP32)
    ones_f32 = pA.tile((128, 64), FP32)

    # ---------------- DMA issues (sync queue; ~600ns per dma_start, so
    # batch each weight tensor into ONE 3D-AP strided descriptor; a
    # host-side pre-shuffle to contiguous DMAs measured ~18us SLOWER).
    # KT jc0 runs first on the PE, so wk's dc0 chunk leads.
    def chunked_w(src, dst, lo_dc=0):
        ap = bass.AP(tensor=src.tensor, offset=src.offset + lo_dc * 128 * JG,
                     ap=[[JG, 128], [128 * JG, DC - lo_dc], [1, JG]])
        nc.sync.dma_start(out=dst[:, lo_dc * JG:DC * JG], in_=ap)

    nc.sync.dma_start(out=wkT_sb[:, 0:JG], in_=wkT[0:128, :])
    nc.sync.dma_start(out=xT_sb[:, 0:512], in_=xT[0:128, 0:512])
    chunked_w(wkT, wkT_sb, lo_dc=1)
    for st in range(1, QT_TILES):
        nc.sync.dma_start(
            out=xT_sb[:, st * 512:(st + 1) * 512],
            in_=xT[0:128, st * 512:(st + 1) * 512])
    for dc in range(1, DC):
        nc.sync.dma_start(out=xT_sb[:, dc * S:(dc + 1) * S],
                          in_=xT[dc * 128:(dc + 1) * 128, :])
    chunked_w(wvT, wvT_sb)
    chunked_w(wqT, wqT_sb)
    bq_ap = bass.AP(tensor=bq.tensor, offset=bq.offset,
                    ap=[[1, 128], [128, 2]])
    nc.sync.dma_start(out=bq_sb[:, 0:2], in_=bq_ap)
    bk_ap = bass.AP(tensor=bk.tensor, offset=bk.offset,
                    ap=[[1, 128], [128, 2]])
    nc.sync.dma_start(out=bk_sb[:, 0:2], in_=bk_ap)
    bv_bcast = bass.AP(tensor=bv.tensor, offset=bv.offset,
                       ap=[[0, 128]] + list(bv.ap))
    nc.gpsimd.dma_start(out=bv_bc, in_=bv_bcast)
    ow_ap = bass.AP(tensor=owT.tensor, offset=owT.offset,
                    ap=[[D, 128], [128 * D, 2], [1, D]])
    nc.sync.dma_start(out=owT_sb[:, 0:2 * D], in_=ow_ap)

    # ones: Vaug's per-head denominator columns + the K=1 broadcast row.
    # memset can't emit bf16-from-float cleanly everywhere; stage fp32 and
    # DVE-copy (converts) into the bf16 tiles. No DMA involved.
    nc.vector.memset(ones_f32, 1.0)
    nc.vector.tensor_copy(Vaug_sb[:, HD::VW], ones_f32)
    ones1_f32 = pA.tile((1, 128), FP32)
    nc.vector.memset(ones1_f32, 1.0)
    nc.vector.tensor_copy(ones1_sb, ones1_f32)

    # ---------------- pre-attention projections ----------------
    # KT jc0: dc-outer over 4 query-tile PSUM banks so the first matmul
    # only needs (wk dc0, x dc0-st) off the wire.
    pss = [ps.tile((128, 512), FP32, tag=["lg", "lg", "op", "op"][st],
                   bufs=2, name=f"kt{st}") for st in range(QT_TILES)]
    for dc in range(DC):
        for st in range(QT_TILES):
            nc.tensor.matmul(
                pss[st],
                wkT_sb[:, dc * JG:dc * JG + 128],
                xT_sb[:, dc * S + st * 512:dc * S + (st + 1) * 512],
                start=(dc == 0), stop=(dc == DC - 1),
            )
            if dc == DC - 1:
                nc.vector.tensor_scalar_add(
                    out=KT_sb[:, st * 512:(st + 1) * 512],
                    in0=pss[st], scalar1=bk_sb[:, 0:1])

    def proj_unit(w_sb, b_sb, dst, jc, st):
        """Generator: one (weight, jc, st) projection chunk, 2 matmuls per
        next(), bias-add folded into the last step."""
        pu = ps.tile((128, 512), FP32, tag="op", bufs=2, name="pu")
        for dc in range(DC):
            nc.tensor.matmul(
                pu,
                w_sb[:, dc * JG + jc * 128:dc * JG + (jc + 1) * 128],
                xT_sb[:, dc * S + st * 512:dc * S + (st + 1) * 512],
                start=(dc == 0), stop=(dc == DC - 1),
            )
            if dc % 2 == 1 and dc < DC - 1:
                yield
        nc.vector.tensor_scalar_add(
            out=dst[:, jc * S + st * 512:jc * S + (st + 1) * 512],
            in0=pu, scalar1=b_sb[:, jc:jc + 1])
        yield

    for _ in proj_unit(wqT_sb, bq_sb, QT_sb, 0, 0):
        pass

    # V chunk: [s-chunk, j-local] into Vaug (stride 65), single strided
    # bias-add. Only chunks 0-2 are emitted pre-attention; block (0,0)
    # self-feeds chunk kc+3 inside its kc loop (the whole V phase ran
    # serially before attention in v3 and delayed the first exp to 45us
    # while ACT idled).
    def v_chunk(sc, tag):
        psv = ps.tile((128, 512), FP32, tag=tag,
                      bufs=1 if tag.startswith("av") else 2, name="psv")
        pv = psv[:, 0:JG]
        for dc in range(DC):
            nc.tensor.matmul(
                pv,
                xT_sb[:, dc * S + sc * 128:dc * S + (sc + 1) * 128],
                wvT_sb[:, dc * JG:(dc + 1) * JG],
                start=(dc == 0), stop=(dc == DC - 1),
            )
        base = sc * 4 * VW
        va = Vaug_sb[:, base:base + 4 * VW]
        nc.vector.tensor_add(
            out=bass.AP(tensor=va.tensor, offset=va.offset,
                        ap=[list(va.ap[0]), [VW, 4], [1, HD]]),
            in0=bass.AP(tensor=pv.tensor, offset=pv.offset,
                        ap=[list(pv.ap[0]), [HD, 4], [1, HD]]),
            in1=bass.AP(tensor=bv_bc.tensor, offset=bv_bc.offset,
                        ap=[list(bv_bc.ap[0]), [HD, 4], [1, HD]]))

    for sc in range(3):
        v_chunk(sc, ["av0", "av1", "op"][sc])

    # jc1 projections stream into the pair-0 kc loops (blocks >= 1) as PE
    # fill-in; QT-jc0 qt1-3 run inline in block (0,0)'s last kc slots so
    # they are emitted (and thus dependency-ordered) before their consumer
    # blocks.
    fill_q = deque()
    for st in range(QT_TILES):
        fill_q.append(proj_unit(wkT_sb, bk_sb, KT_sb, 1, st))
    for st in range(QT_TILES):
        fill_q.append(proj_unit(wqT_sb, bq_sb, QT_sb, 1, st))

    def fill_step():
        while fill_q:
            try:
                next(fill_q[0])
                return
            except StopIteration:
                fill_q.popleft()

    def outproj_unit(st, it):
        """Generator: one [128,512] out-projection tile; 2 matmuls then
        cast+DMA, one next() each."""
        po = ps.tile((128, 512), FP32, tag="op", bufs=2, name="po")
        for jc in range(2):
            nc.tensor.matmul(
                po,
                attn_outT_sb[:, jc * S + st * 128:jc * S + st * 128 + 128],
                owT_sb[:, jc * D + it * 512:jc * D + (it + 1) * 512],
                start=(jc == 0), stop=(jc == 1))
        yield
        ost = pB.tile((128, 512), FP16, tag="ost", bufs=4, name="ost")
        nc.vector.tensor_copy(ost, po)
        nc.sync.dma_start(
            out=out[st * 128:(st + 1) * 128, it * 512:(it + 1) * 512],
            in_=ost)
        yield

    def queue_outproj(qt):
        for st in range(4 * qt, 4 * qt + 4):
            for it in range(2):
                fill_q.append(outproj_unit(st, it))

    # ---------------- attention (pair-outer) ----------------
    # Each block's normalize (bc matmuls + recip + muls) is DEFERRED into
    # the next block's first kc slot: emitting it at block end puts the bc
    # matmuls (which wait a ~1.5us DVE copy chain) ahead of the next
    # block's logits in the PE stream and stalls ACT ~2.5us per boundary.
    def make_normalize(avs0, avs1, d0, d1, base, chunked=False):
        def emit():
            bc0 = ps.tile((128, 512), FP32, tag="op", bufs=2, name="bc0")
            nc.tensor.matmul(bc0, ones1_sb, d0, start=True, stop=True)
            bc1 = ps.tile((128, 512), FP32, tag="op", bufs=2, name="bc1")
            nc.tensor.matmul(bc1, ones1_sb, d1, start=True, stop=True)
            # reciprocal_approx_fast is only safe on full-tile,
            # partition-base-0 APs (sub-ranges mis-evaluate -> NaN)
            rcs0 = pB.tile((128, 512), FP32, tag="rcs", bufs=2, name="rcs0")
            nc.vector.reciprocal_approx_fast(rcs0, bc0)
            rcs1 = pB.tile((128, 512), FP32, tag="rcs", bufs=2, name="rcs1")
            nc.vector.reciprocal_approx_fast(rcs1, bc1)
            chunks = ((0, 128, (12,)), (128, 512, (13, 14, 15))) if chunked \
                else ((0, 512, ()),)
            nu = 0
            for lo, hi, sts in chunks:
                nc.vector.tensor_mul(
                    out=attn_outT_sb[0:HD, base + lo:base + hi],
                    in0=avs0[0:HD, lo:hi], in1=rcs0[0:HD, lo:hi])
                nc.vector.tensor_mul(
                    out=attn_outT_sb[HD:128, base + lo:base + hi],
                    in0=avs1[0:HD, lo:hi], in1=rcs1[0:HD, lo:hi])
                # tail out-projection: rotate over 4 PSUM slots (the lg
                # banks are free once the exps are done) and alternate
                # casts between DVE and the now-idle ACT so the tail is
                # matmul-paced, not cast-paced.
                for st in sts:
                    for it in range(2):
                        po = ps.tile((128, 512), FP32,
                                     tag=["op", "lg"][nu % 2], bufs=2,
                                     name="pof")
                        for jc in range(2):
                            nc.tensor.matmul(
                                po,
                                attn_outT_sb[:, jc * S + st * 128:
                                             jc * S + st * 128 + 128],
                                owT_sb[:, jc * D + it * 512:
                                       jc * D + (it + 1) * 512],
                                start=(jc == 0), stop=(jc == 1))
                        ost = pB.tile((128, 512), FP16, tag="ost", bufs=4,
                                      name="ost")
                        if nu % 2 == 0:
                            nc.vector.tensor_copy(ost, po)
                        else:
                            nc.scalar.activation(
                                ost, po, mybir.ActivationFunctionType.Copy)
                        nc.sync.dma_start(
                            out=out[st * 128:(st + 1) * 128,
                                    it * 512:(it + 1) * 512],
                            in_=ost)
                        nu += 1
        return emit

    norm_pend = None
    for pair in range(2):
        for qt in range(QT_TILES):
            av0 = ps.tile((128, 512), FP32, tag="av0", bufs=1, name="av0")
            av1 = ps.tile((128, 512), FP32, tag="av1", bufs=1, name="av1")
            qcol = pair * S + qt * 512
            pend = None

            def emit_av(kc, at):
                for h, avp, off in ((2 * pair, av0, 0),
                                    (2 * pair + 1, av1, 512)):
                    nc.tensor.matmul(
                        avp[0:VW, :],
                        Vaug_sb[:, kc * 4 * VW + h * VW:
                                kc * 4 * VW + (h + 1) * VW],
                        at[:, off:off + 512],
                        start=(kc == 0), stop=(kc == KC - 1))

            # kc loop, software-pipelined one stage: fill-in matmuls go
            # between the logits matmuls (which never stall) and the AV
            # matmuls for kc-1 (which wait on the exp).
            for kc in range(KC):
                lg = ps.tile((128, 1024), FP32, tag="lg", bufs=2, name="lg")
                kcol = pair * S + kc * 128
                nc.tensor.matmul(
                    lg[:, 0:512],
                    KT_sb[0:64, kcol:kcol + 128],
                    QT_sb[0:64, qcol:qcol + 512],
                    start=True, stop=True, tile_position=(0, 0))
                nc.tensor.matmul(
                    lg[:, 512:1024],
                    KT_sb[64:128, kcol:kcol + 128],
                    QT_sb[64:128, qcol:qcol + 512],
                    start=True, stop=True, tile_position=(64, 0))
                if pair == 0 and qt == 0:
                    # block (0,0) self-feeds: V chunk kc+3 just-in-time for
                    # the AV matmuls, then the remaining jc0 Q projections.
                    if kc < KC - 3:
                        v_chunk(kc + 3, "op")
                    else:
                        for _ in proj_unit(wqT_sb, bq_sb, QT_sb, 0,
                                           kc - (KC - 3) + 1):
                            pass
                else:
                    fill_step()
                if pend is not None:
                    emit_av(*pend)
                at = pB.tile((128, 1024), BF16, tag="at", bufs=3, name="at")
                nc.scalar.activation(at, lg, EXP)
                pend = (kc, at)
                if kc == 0:
                    if norm_pend is not None:
                        norm_pend()
                        norm_pend = None
                    if pair == 1 and qt > 0:
                        queue_outproj(qt - 1)
            emit_av(*pend)

            # denominator rows first (the bc matmuls need them soonest),
            # then the av rows to SBUF fp32 -- frees the av banks so the
            # next block's first AV matmul doesn't WAR-wait the normalize.
            d0 = pB.tile((1, 512), BF16, tag="d0", bufs=2, name="d0")
            nc.vector.tensor_copy(d0, av0[HD:HD + 1, :])
            d1 = pB.tile((1, 512), BF16, tag="d1", bufs=2, name="d1")
            nc.vector.tensor_copy(d1, av1[HD:HD + 1, :])
            avs0 = pB.tile((VW, 512), FP32, tag="avs0", bufs=2, name="avs0")
            nc.vector.tensor_copy(avs0, av0[0:VW, :])
            avs1 = pB.tile((VW, 512), FP32, tag="avs1", bufs=2, name="avs1")
            nc.vector.tensor_copy(avs1, av1[0:VW, :])
            base = pair * S + qt * 512
            norm_pend = make_normalize(
                avs0, avs1, d0, d1, base,
                chunked=(pair == 1 and qt == QT_TILES - 1))

    # tail: drain leftover fill units, then the last block's normalize with
    # its out-projection st-units interleaved
    while fill_q:
        fill_step()
    norm_pend()


_NC = None


def _build_nc():
    global _NC
    if _NC is not None:
        return _NC
    nc = bacc.Bacc("TRN2", target_bir_lowering=False, debug=False,
                   num_devices=NCORES)
    xT = nc.dram_tensor("xT", [D, S], FP16, kind="ExternalInput").ap()
    wqT = nc.dram_tensor("wqT", [D, JG], FP16, kind="ExternalInput").ap()
    wkT = nc.dram_tensor("wkT", [D, JG], FP16, kind="ExternalInput").ap()
    wvT = nc.dram_tensor("wvT", [D, JG], FP16, kind="ExternalInput").ap()
    bq = nc.dram_tensor("bq", [JG], FP32, kind="ExternalInput").ap()
    bk = nc.dram_tensor("bk", [JG], FP32, kind="ExternalInput").ap()
    bv = nc.dram_tensor("bv", [JG], FP32, kind="ExternalInput").ap()
    owT = nc.dram_tensor("owT", [JG, D], FP16, kind="ExternalInput").ap()
    out = nc.dram_tensor("out", [S, D], FP16, kind="ExternalOutput").ap()
    with tile.TileContext(nc) as tc:
        mha_core_kernel(tc, out, xT, wqT, wkT, wvT, bq, bk, bv, owT)
    nc.compile()
    _NC = nc
    return nc


def _in_maps(x, kqv_w, kqv_b, out_w):
    maps = []
    xT16 = [np.ascontiguousarray(x[b].T.astype(np.float16)) for b in range(B)]
    for c in range(NCORES):
        b, g = divmod(c, 4)
        sl = slice(g * JG, (g + 1) * JG)
        maps.append({
            "xT": xT16[b],
            "wqT": np.ascontiguousarray(kqv_w[0 * D:1 * D][sl].T.astype(np.float16)),
            "wkT": np.ascontiguousarray(kqv_w[1 * D:2 * D][sl].T.astype(np.float16)),
            "wvT": np.ascontiguousarray(kqv_w[2 * D:3 * D][sl].T.astype(np.float16)),
            "bq": np.ascontiguousarray(kqv_b[0 * D:1 * D][sl]),
            "bk": np.ascontiguousarray(kqv_b[1 * D:2 * D][sl]),
            "bv": np.ascontiguousarray(kqv_b[2 * D:3 * D][sl]),
            "owT": np.ascontiguousarray(out_w[:, sl].T.astype(np.float16)),
        })
    return maps


def run_spmd(x, kqv_w, kqv_b, out_w, out_b, trace=False, tmpdir=None):
    nc = _build_nc()
    res = run_bass_kernel_spmd(nc, _in_maps(x, kqv_w, kqv_b, out_w),
                               list(range(NCORES)), tmpdir=tmpdir, trace=trace)
    parts = [np.asarray(res.results[c]["out"], dtype=np.float32)
             for c in range(NCORES)]
    full = np.stack([
        parts[4 * b] + parts[4 * b + 1] + parts[4 * b + 2] + parts[4 * b + 3]
        + out_b[None, :].astype(np.float32)
        for b in range(B)
    ])
    return full, res


def kernel(**inputs):
    x = np.asarray(inputs["x"], dtype=np.float32)
    kqv_w = np.asarray(inputs["kqv_w"], dtype=np.float32)
    kqv_b = np.asarray(inputs["kqv_b"], dtype=np.float32)
    out_w = np.asarray(inputs["out_w"], dtype=np.float32)
    out_b = np.asarray(inputs["out_b"], dtype=np.float32)
    full, _ = run_spmd(x, kqv_w, kqv_b, out_w, out_b)
    return full
